# revision 42
# baseline (speedup 1.0000x reference)
import sys
import numpy as np

sys.path.insert(0, "/opt/trn_rl_repo")

import concourse.bass as bass  # noqa: E402
import concourse.tile as tile  # noqa: E402
from concourse import bacc, mybir  # noqa: E402
from concourse.ap import AP  # noqa: E402
from concourse.bass_utils import run_bass_kernel_spmd  # noqa: E402
import ml_dtypes  # noqa: E402

BF16 = mybir.dt.bfloat16
F32 = mybir.dt.float32
DIM = 70
HW = DIM * DIM  # 4900
CUBE = DIM * HW  # 343000

_CACHE = {}


def _build():
    nc = bacc.Bacc("TRN2", target_bir_lowering=False, debug=False, num_devices=8)
    a1_d = nc.dram_tensor("a1", [48, 6 * 70], BF16, kind="ExternalInput")
    kr_d = nc.dram_tensor("kr", [48, 6 * HW], BF16, kind="ExternalInput")
    w1_d = nc.dram_tensor("w1", [108, 384], BF16, kind="ExternalInput")
    w2_d = nc.dram_tensor("w2", [128, 1152], BF16, kind="ExternalInput")
    w3a_d = nc.dram_tensor("w3a", [128, 1152], BF16, kind="ExternalInput")
    w3b_d = nc.dram_tensor("w3b", [64, 1152], BF16, kind="ExternalInput")
    w4_d = nc.dram_tensor("w4", [128, 27 * 256], BF16, kind="ExternalInput")
    f1_d = nc.dram_tensor("f1", [128, 16 * 1024], BF16, kind="ExternalInput")
    f2_d = nc.dram_tensor("f2", [128, 8 * 29], BF16, kind="ExternalInput")
    b1_d = nc.dram_tensor("b1", [128, 1], F32, kind="ExternalInput")
    b2_d = nc.dram_tensor("b2", [128, 1], F32, kind="ExternalInput")
    b3_d = nc.dram_tensor("b3", [128, 1], F32, kind="ExternalInput")
    b4_d = nc.dram_tensor("b4", [128, 2], F32, kind="ExternalInput")
    fb1_d = nc.dram_tensor("fb1", [128, 8], F32, kind="ExternalInput")
    fb2_d = nc.dram_tensor("fb2", [29, 1], F32, kind="ExternalInput")
    y_d = nc.dram_tensor("y", [29], F32, kind="ExternalOutput")

    Relu = mybir.ActivationFunctionType.Relu
    Copy = mybir.ActivationFunctionType.Copy
    amax = mybir.AluOpType.max
    aadd = mybir.AluOpType.add
    XY = mybir.AxisListType.XY

    with tile.TileContext(nc, pool_alloc_mode="queue") as tc:
        with (
            tc.tile_pool(name="const", bufs=1) as constp,
        ):
            w1 = constp.tile([108, 384], BF16)
            w2 = constp.tile([128, 1152], BF16)
            b1 = constp.tile([128, 1], F32)
            b2 = constp.tile([128, 1], F32)
            b3 = constp.tile([128, 1], F32)
            b4 = constp.tile([128, 2], F32)
            fb1 = constp.tile([128, 8], F32)
            fb2 = constp.tile([29, 1], F32)

            # ---------------- blur (sparse rank-n expansion) ----------------
            # cube[e][a,p,q] = sum_n A1[n,a] * KR[n,(p,q)]
            # A1[n,:] = G_e[:,i_n];  KR[n] = outer(G_e[:,j_n], G_e[:,k_n])
            # cube lives in SBUF: CB[z, e*4900 + y*70 + x]
            h2p = tc.alloc_tile_pool(name="h2p", bufs=1)
            H2T = [h2p.tile([128, 1156], BF16, name=f"H2T{i}")
                   for i in range(16)]
            h3p = tc.alloc_tile_pool(name="h3p", bufs=1)
            H3D = h3p.tile([128, 16 * 256], BF16)
            cbp = tc.alloc_tile_pool(name="cbp", bufs=1)
            CB = cbp.tile([70, 6 * HW], BF16)
            with (
                tc.tile_pool(name="xinp", bufs=1) as xinp,
                tc.tile_pool(name="bps", bufs=4, space="PSUM") as bps,
            ):
                a1 = xinp.tile([48, 6 * 70], BF16)
                nc.sync.dma_start(a1[:], a1_d[:])
                kr = xinp.tile([48, 6 * HW], BF16)
                for hh in range(12):
                    nc.sync.dma_start(kr[:, hh * 2450:(hh + 1) * 2450],
                                      kr_d[:, hh * 2450:(hh + 1) * 2450])
                nc.sync.dma_start(w1[:], w1_d[:])
                nc.sync.dma_start(w2[:], w2_d[:])
                nc.sync.dma_start(b1[:], b1_d[:])
                nc.sync.dma_start(b2[:], b2_d[:])
                nc.sync.dma_start(b3[:], b3_d[:])
                nc.sync.dma_start(b4[:], b4_d[:])
                nc.sync.dma_start(fb1[:], fb1_d[:])
                nc.sync.dma_start(fb2[:], fb2_d[:])

                eng_load = [0.0, 0.0]  # act, dve
                for e in range(6):
                    for g in range(5):
                        ps = bps.tile([70, 1024], F32, tag="ps", name="bps")
                        for h in range(2):
                            off = e * HW + g * 980 + h * 490
                            nc.tensor.matmul(ps[:, h * 512:h * 512 + 490],
                                             a1[:, e * 70:(e + 1) * 70],
                                             kr[:, off:off + 490])
                        src_ = ps[:].rearrange("p (h r) -> p h r", h=2)[:, :, 0:490]
                        dst = CB[:, e * HW + g * 980:e * HW + (g + 1) * 980]
                        dst = dst.rearrange("p (h r) -> p h r", h=2)
                        if eng_load[0] + 959 <= eng_load[1] + 1146:
                            nc.scalar.activation(dst, src_, Copy)
                            eng_load[0] += 959
                        else:
                            nc.vector.tensor_copy(dst, src_)
                            eng_load[1] += 1146
            # ---------------- conv1 (+pool+relu) -> H2T tiles ----------------
            # K = (dy3, dzw6, e6) = 108, M = (zb4, ch32) = 128, accumulate dx.
            # H2T[t] : [128 = (dzw4, ch32), 34*34], planes 2t..2t+3
            with (
                tc.tile_pool(name="ring", bufs=5) as ringp,
                tc.tile_pool(name="c1ps", bufs=3, space="PSUM") as c1ps,
                tc.tile_pool(name="c1s", bufs=6) as c1sp,
                tc.tile_pool(name="c1u", bufs=6) as c1up,
                tc.tile_pool(name="c2ps", bufs=1, space="PSUM") as c2ps,
                tc.tile_pool(name="c2s", bufs=2) as c2sp,
            ):
                def conv2_tile(t):
                    hv = H2T[t][:].rearrange("p (y x) -> p y x", y=34)
                    ps = c2ps.tile([128, 1024], F32, tag="ps2")
                    for h in range(2):
                        for st in range(9):
                            dy, dx = st // 3, st % 3
                            rhs = hv[:, h * 16 + dy:h * 16 + dy + 16, dx:dx + 32]
                            nc.tensor.matmul(
                                ps[:, h * 512:(h + 1) * 512],
                                w2[:, st * 128:(st + 1) * 128], rhs,
                                start=(st == 0), stop=(st == 8))
                    S2 = c2sp.tile([128, 1024], BF16, tag="S2")
                    nc.scalar.activation(S2[:], ps[:], Relu, bias=b2[:])
                    sv = S2[:].rearrange("p (h yp wy x) -> p h yp wy x",
                                         h=2, yp=8, wy=2)
                    Y2 = c2sp.tile([128, 512], BF16, tag="Y2")
                    yv = Y2[:].rearrange("p (h yp x) -> p h yp x", h=2, yp=8)
                    nc.vector.tensor_tensor(yv[:], sv[:, :, :, 0, :],
                                            sv[:, :, :, 1, :], amax)
                    yvp = Y2[:].rearrange("p (h yp xp wx) -> p h yp xp wx",
                                          h=2, yp=8, wx=2)
                    U2 = c2sp.tile([128, 256], BF16, tag="U2")
                    uv = U2[:].rearrange("p (h yp xp) -> p h yp xp", h=2, yp=8)
                    nc.vector.tensor_tensor(uv[:], yvp[:, :, :, :, 0],
                                            yvp[:, :, :, :, 1], amax)
                    V2c = c2sp.tile([64, 256], BF16, tag="V2c")
                    nc.sync.dma_start(V2c[:], U2[64:128, :])
                    h3s = H3D[0:64, t * 256:(t + 1) * 256]
                    nc.vector.tensor_tensor(h3s, U2[0:64, :], V2c[:], amax)
                    if t >= 1:
                        nc.sync.dma_start(
                            H3D[64:128, (t - 1) * 256:t * 256], h3s)

                slices = {}

                cbv = CB[:].rearrange("p (e f) -> p e f", e=6)

                def load_tile(w):
                    z0 = 4 * w
                    t = ringp.tile([108, HW], BF16, tag="sl")
                    for dy in range(3):
                        L = HW - 70 * dy
                        src = cbv[z0:z0 + 6, :, 70 * dy:HW]
                        nc.sync.dma_start(t[dy * 36:(dy + 1) * 36, 0:L], src)
                    slices[w] = t

                load_tile(0)
                load_tile(1)
                load_tile(2)
                for w in range(17):
                    if w + 3 < 17:
                        load_tile(w + 3)
                    T = slices.pop(w)
                    U = c1up.tile([128, 1156], BF16, tag="U")
                    # chunk pairs cp=0..4 (chunks 0..9, ny=6); tail chunks 10, 11
                    for cp in range(5):
                        ps = c1ps.tile([128, 1024], F32, tag="ps")
                        for h in range(2):
                            c = cp * 2 + h
                            tv = T[:].rearrange("p (y x) -> p y x", y=70)
                            for dx in range(3):
                                rhs = tv[:, 6 * c:6 * c + 6, dx:dx + 68]
                                nc.tensor.matmul(
                                    ps[:, h * 512:h * 512 + 408],
                                    w1[:, dx * 128:(dx + 1) * 128], rhs,
                                    start=(dx == 0), stop=(dx == 2))
                        S = c1sp.tile([128, 816], BF16, tag="S")
                        psrc = ps[:].rearrange("p (h r) -> p h r",
                                               h=2)[:, :, 0:408]
                        sdst = S[:].rearrange("p (h r) -> p h r", h=2)
                        nc.scalar.activation(sdst, psrc, Relu, bias=b1[:])
                        # y-pool: (c2, yp3, 68x)
                        Yt = c1sp.tile([128, 408], BF16, tag="Y")
                        yv = Yt[:].rearrange("p (c yp x) -> p c yp x",
                                             c=2, yp=3)
                        svp = S[:].rearrange("p (c yp wy x) -> p c yp wy x",
                                             c=2, yp=3, wy=2)
                        nc.vector.tensor_tensor(yv[:], svp[:, :, :, 0, :],
                                                svp[:, :, :, 1, :], amax)
                        # x-pool: (c2, yp3, xp34)
                        yvp = Yt[:].rearrange(
                            "p (c yp xp wx) -> p c yp xp wx",
                            c=2, yp=3, wx=2)
                        uo = U[:, cp * 204:(cp + 1) * 204].rearrange(
                            "p (c yp xp) -> p c yp xp", c=2, yp=3)
                        nc.vector.tensor_tensor(uo[:], yvp[:, :, :, :, 0],
                                                yvp[:, :, :, :, 1], amax)
                    # tail: chunks 10 (ny=6), 11 (ny=2) in one psum tile
                    ps = c1ps.tile([128, 1024], F32, tag="ps")
                    tv = T[:].rearrange("p (y x) -> p y x", y=70)
                    for h, (c, ny) in enumerate([(10, 6), (11, 2)]):
                        for dx in range(3):
                            rhs = tv[:, 6 * c:6 * c + ny, dx:dx + 68]
                            nc.tensor.matmul(
                                ps[:, h * 512:h * 512 + ny * 68],
                                w1[:, dx * 128:(dx + 1) * 128], rhs,
                                start=(dx == 0), stop=(dx == 2))
                    S = c1sp.tile([128, 816], BF16, tag="S")
                    nc.scalar.activation(S[:, 0:408], ps[:, 0:408], Relu, bias=b1[:])
                    nc.scalar.activation(S[:, 408:544], ps[:, 512:648], Relu,
                                         bias=b1[:])
                    # c10: y/x pool
                    Yt = c1sp.tile([128, 408], BF16, tag="Y")
                    s10 = S[:, 0:408].rearrange("p (yp wy x) -> p yp wy x",
                                                yp=3, wy=2)
                    y10 = Yt[:, 0:204].rearrange("p (yp x) -> p yp x", yp=3)
                    nc.vector.tensor_tensor(y10[:], s10[:, :, 0, :],
                                            s10[:, :, 1, :], amax)
                    yv10 = Yt[:, 0:204].rearrange("p (yp xp wx) -> p yp xp wx",
                                                  yp=3, wx=2)
                    u10 = U[:, 1020:1122].rearrange("p (yp xp) -> p yp xp", yp=3)
                    nc.vector.tensor_tensor(u10[:], yv10[:, :, :, 0],
                                            yv10[:, :, :, 1], amax)
                    # c11: ny=2 -> 1 yp row
                    s11 = S[:, 408:544].rearrange("p (wy x) -> p wy x", wy=2)
                    y11 = Yt[:, 204:272]
                    nc.vector.tensor_tensor(y11, s11[:, 0, :], s11[:, 1, :], amax)
                    yv11 = Yt[:, 204:272].rearrange("p (xp wx) -> p xp wx", wx=2)
                    nc.vector.tensor_tensor(U[:, 1122:1156], yv11[:, :, 0],
                                            yv11[:, :, 1], amax)
                    # z-pool: U rows = (parity, zp, ch); planes pair across
                    # the 64-partition halves -> 1 copy + 1 tt
                    V2 = c1up.tile([64, 1156], BF16, tag="V2")
                    nc.sync.dma_start(V2[:], U[64:128, :])
                    if w < 16:
                        nc.vector.tensor_tensor(H2T[w][0:64, :], U[0:64, :],
                                                V2[:], amax)
                        if w >= 1:
                            nc.sync.dma_start(H2T[w - 1][64:128, :],
                                              H2T[w][0:64, :])
                    else:
                        nc.vector.tensor_tensor(H2T[15][64:128, :], U[0:64, :],
                                                V2[:], amax)
                    if w >= 3:
                        conv2_tile(w - 3)

                for t in range(14, 16):
                    conv2_tile(t)
            cbp.release()
            fcp = tc.alloc_tile_pool(name="fcp", bufs=1)
            w3a = fcp.tile([128, 1152], BF16)
            nc.sync.dma_start(w3a[:], w3a_d[:])
            w3b = fcp.tile([64, 1152], BF16)
            nc.sync.dma_start(w3b[:], w3b_d[:])
            w4 = fcp.tile([128, 27 * 256], BF16)
            nc.sync.dma_start(w4[:], w4_d[:])
            f1 = fcp.tile([128, 16 * 1024], BF16)
            nc.sync.dma_start(f1[:], f1_d[:])
            f2 = fcp.tile([128, 8 * 29], BF16)
            nc.sync.dma_start(f2[:], f2_d[:])
            # ---------------- conv3 (+pool+relu) -> H4 ----------------
            h4p = tc.alloc_tile_pool(name="h4p", bufs=1)
            H4 = h4p.tile([128, 343], BF16)
            with (
                tc.tile_pool(name="c3ps", bufs=6, space="PSUM") as c3ps,
                tc.tile_pool(name="c3u", bufs=1) as c3up,
            ):
                U3 = c3up.tile([128, 686], BF16)
                h3v = H3D[:].rearrange("p (t y x) -> p t y x", t=16, y=16)
                for z in range(14):
                    ps = c3ps.tile([128, 196], F32, tag="ps")
                    for st in range(9):
                        dy, dx = st // 3, st % 3
                        rhs = h3v[:, z, dy:dy + 14, dx:dx + 14]
                        nc.tensor.matmul(ps[:], w3a[:, st * 128:(st + 1) * 128],
                                         rhs, start=(st == 0), stop=False)
                    for st in range(9):
                        dy, dx = st // 3, st % 3
                        rhs = h3v[0:64, z + 2, dy:dy + 14, dx:dx + 14]
                        nc.tensor.matmul(ps[:], w3b[:, st * 128:(st + 1) * 128],
                                         rhs, start=False, stop=(st == 8))
                    pv = ps[:].rearrange("p (yp wy xp wx) -> p yp xp wy wx",
                                         yp=7, wy=2, wx=2)
                    uo = U3[:, z * 49:(z + 1) * 49].rearrange(
                        "p (yp xp) -> p yp xp", yp=7)
                    nc.vector.tensor_reduce(uo[:], pv[:], XY, amax)
                u3v = U3[:].rearrange("p (zp wz yx) -> p zp wz yx", zp=7, wz=2)
                h4v = H4[:].rearrange("p (zp yx) -> p zp yx", zp=7)
                nc.vector.tensor_tensor(h4v[:], u3v[:, :, 0, :], u3v[:, :, 1, :],
                                        amax)
                nc.scalar.activation(H4[:], H4[:], Relu, bias=b3[:])

            # ---------------- conv4 + fc ----------------
            with (
                tc.tile_pool(name="c4ps", bufs=2, space="PSUM") as c4ps,
                tc.tile_pool(name="c4tmp", bufs=8) as c4tmp,
            ):
                h4r = H4[:].rearrange("p (z y x) -> p z y x", z=7, y=7)
                v = c4tmp.tile([128, 16], BF16, tag="v")
                for mt in range(2):
                    ps = c4ps.tile([128, 125], F32, tag="ps")
                    for t in range(27):
                        dz, dy, dx = t // 9, (t // 3) % 3, t % 3
                        rhs = h4r[:, dz:dz + 5, dy:dy + 5, dx:dx + 5]
                        nc.tensor.matmul(
                            ps[:],
                            w4[:, t * 256 + mt * 128:t * 256 + (mt + 1) * 128],
                            rhs, start=(t == 0), stop=(t == 26))
                    pr0 = ps[:].rearrange("p (z y x) -> p z y x", z=5, y=5)
                    pr = pr0[:, :, :, 0:4].rearrange(
                        "p z y (xp two) -> p (z y) xp two", two=2)
                    xt = c4tmp.tile([128, 50], F32, tag="xt")
                    xtr = xt[:].rearrange("p (zy x) -> p zy x", x=2)
                    nc.vector.tensor_reduce(xtr[:], pr[:],
                                            mybir.AxisListType.X, amax)
                    x20 = xt[:].rearrange("p (z y x) -> p z y x", z=5, y=5)
                    x2 = x20[:, :, 0:4, :].rearrange(
                        "p z (yp two) x -> p z yp two x", two=2)
                    yt = c4tmp.tile([128, 20], F32, tag="yt")
                    ytr = yt[:].rearrange("p (z y x) -> p z y x", z=5, y=2)
                    nc.vector.tensor_tensor(ytr[:], x2[:, :, :, 0, :],
                                            x2[:, :, :, 1, :], amax)
                    y2r0 = yt[:].rearrange("p (z yx) -> p z yx", z=5)
                    y2r = y2r0[:, 0:4, :].rearrange(
                        "p (zp two) yx -> p zp two yx", two=2)
                    zt = c4tmp.tile([128, 8], F32, tag="zt")
                    ztr = zt[:].rearrange("p (z yx) -> p z yx", z=2)
                    nc.vector.tensor_tensor(ztr[:], y2r[:, :, 0, :],
                                            y2r[:, :, 1, :], amax)
                    nc.scalar.activation(v[:, mt * 8:(mt + 1) * 8], zt[:],
                                         Relu, bias=b4[:, mt:mt + 1])
                # fc1
                ps5 = c4ps.tile([128, 8], F32, tag="fc1")
                for m in range(8):
                    for kt in range(16):
                        nc.tensor.matmul(
                            ps5[:, m:m + 1],
                            f1[:, kt * 1024 + m * 128:kt * 1024 + (m + 1) * 128],
                            v[:, kt:kt + 1],
                            start=(kt == 0), stop=(kt == 15))
                y1s = c4tmp.tile([128, 8], F32, tag="y1a")
                nc.vector.tensor_tensor(y1s[:], ps5[:], fb1[:], aadd)
                y1b = c4tmp.tile([128, 8], BF16, tag="y1b")
                nc.vector.tensor_scalar_max(y1b[:], y1s[:], 0.0)
                # fc2
                ps6 = c4ps.tile([29, 1], F32, tag="fc2")
                for kt in range(8):
                    nc.tensor.matmul(ps6[:], f2[:, kt * 29:(kt + 1) * 29],
                                     y1b[:, kt:kt + 1],
                                     start=(kt == 0), stop=(kt == 7))
                yout = c4tmp.tile([29, 1], F32, tag="yo")
                nc.vector.tensor_tensor(yout[:], ps6[:], fb2[:], aadd)
                nc.sync.dma_start(AP(y_d, 0, [[1, 29], [1, 1]]), yout[:])
            h4p.release()
            fcp.release()
            h3p.release()
            h2p.release()
    nc.compile()
    return nc


def _prep(inputs):
    x = np.asarray(inputs["x"], np.float32)
    sigma = np.asarray(inputs["sigma"], np.float32)
    coords = np.arange(DIM, dtype=np.float32) - DIM / 2.0
    idx = np.arange(DIM, dtype=np.float32)
    d2 = (coords[:, None] - idx[None, :]) ** 2
    G = np.exp(-d2[None] / (2.0 * sigma[:, None, None] ** 2))  # [6, a, i]
    gt = np.ascontiguousarray(G.transpose(0, 2, 1))            # [6, i, a]
    gt_dev = np.zeros((70, 6 * 70), np.float32)
    for e in range(6):
        gt_dev[:, e * 70:(e + 1) * 70] = gt[e]

    w1 = np.asarray(inputs["conv1_w"], np.float32)  # [32,6,3,3,3] (o,i,dz,dy,dx)
    w1n = np.zeros((108, 384), np.float32)
    for dy in range(3):
        for dzw in range(6):
            for e in range(6):
                r = dy * 36 + dzw * 6 + e
                for dx in range(3):
                    for zb in range(4):
                        dz = dzw - zb
                        if 0 <= dz <= 2:
                            col = dx * 128 + (zb & 1) * 64 + (zb >> 1) * 32
                            w1n[r, col:col + 32] = w1[:, e, dz, dy, dx]
    w2 = np.asarray(inputs["conv2_w"], np.float32)  # [64,32,3,3,3]
    w2n = np.zeros((128, 1152), np.float32)
    for dzw in range(4):
        for cin in range(32):
            r = dzw * 32 + cin
            for t in range(9):
                dy, dx = t // 3, t % 3
                for g in range(2):
                    dz = dzw - g
                    if 0 <= dz <= 2:
                        w2n[r, t * 128 + g * 64:t * 128 + g * 64 + 64] = \
                            w2[:, cin, dz, dy, dx]
    w3 = np.asarray(inputs["conv3_w"], np.float32)  # [128,64,3,3,3]
    w3a = np.zeros((128, 1152), np.float32)
    w3b = np.zeros((64, 1152), np.float32)
    for t in range(9):
        dy, dx = t // 3, t % 3
        for dzw in range(2):
            w3a[dzw * 64:(dzw + 1) * 64, t * 128:(t + 1) * 128] = \
                w3[:, :, dzw, dy, dx].T
        w3b[:, t * 128:(t + 1) * 128] = w3[:, :, 2, dy, dx].T
    w4 = np.asarray(inputs["conv4_w"], np.float32)  # [256,128,3,3,3]
    w4_dev = np.zeros((128, 27 * 256), np.float32)
    for t in range(27):
        dz, dy, dx = t // 9, (t // 3) % 3, t % 3
        for mt in range(2):
            w4_dev[:, t * 256 + mt * 128:t * 256 + (mt + 1) * 128] = \
                w4[mt * 128:(mt + 1) * 128, :, dz, dy, dx].T
    fc1w = np.asarray(inputs["fc1_w"], np.float32)  # [1024, 2048]
    f1_dev = np.zeros((128, 16 * 1024), np.float32)
    for kt in range(16):
        mt, vox = kt // 8, kt % 8
        for p in range(128):
            f1_dev[p, kt * 1024:(kt + 1) * 1024] = fc1w[:, (mt * 128 + p) * 8 + vox]
    fc2w = np.asarray(inputs["fc2_w"], np.float32)  # [29, 1024]
    f2_dev = np.zeros((128, 8 * 29), np.float32)
    for kt in range(8):
        f2_dev[:, kt * 29:(kt + 1) * 29] = fc2w[:, kt * 128:(kt + 1) * 128].T

    bf = lambda a: a.astype(ml_dtypes.bfloat16)  # noqa: E731
    common = dict(
        w1=bf(w1n), w2=bf(w2n), w3a=bf(w3a), w3b=bf(w3b),
        w4=bf(w4_dev), f1=bf(f1_dev), f2=bf(f2_dev),
        b1=np.tile(np.asarray(inputs["conv1_b"], np.float32), 4).reshape(128, 1),
        b2=np.tile(np.asarray(inputs["conv2_b"], np.float32), 2).reshape(128, 1),
        b3=np.asarray(inputs["conv3_b"], np.float32).reshape(128, 1),
        b4=np.asarray(inputs["conv4_b"], np.float32).reshape(2, 128).T.copy(),
        fb1=np.asarray(inputs["fc1_b"], np.float32).reshape(8, 128).T.copy(),
        fb2=np.asarray(inputs["fc2_b"], np.float32).reshape(29, 1),
    )
    in_maps = []
    for b in range(8):
        a1_dev = np.zeros((48, 6 * 70), np.float32)
        kr_dev = np.zeros((48, 6 * HW), np.float32)
        for e in range(6):
            idx = np.argwhere(x[b, e] != 0)
            n = len(idx)
            assert n <= 48, f"atom count {n} exceeds kernel capacity"
            gte = gt[e]  # [v, a] = G[e, a, v].T
            a1_dev[:n, e * 70:(e + 1) * 70] = gte[idx[:, 0]]
            kr_dev[:n, e * HW:(e + 1) * HW] = (
                gte[idx[:, 1]][:, :, None] * gte[idx[:, 2]][:, None, :]
            ).reshape(n, HW)
        m = dict(common)
        m["a1"] = bf(a1_dev)
        m["kr"] = bf(kr_dev)
        in_maps.append(m)
    return in_maps


def kernel(**inputs):
    if "nc" not in _CACHE:
        _CACHE["nc"] = _build()
    nc = _CACHE["nc"]
    in_maps = _prep(inputs)
    res = run_bass_kernel_spmd(nc, in_maps, core_ids=list(range(8)))
    out = np.stack([res.results[b]["y"] for b in range(8)], axis=0)
    return out.astype(np.float32)


if __name__ == "__main__":
    pass
